# revision 93
# baseline (speedup 1.0000x reference)
"""Trainium2 Bass kernel for MemoryAsContextTransformer segmented attention.

Reference computation (per full input):
  h   = rmsnorm(x, gamma)                      [B=2, S=4096, D=1024]
  qkv = h @ w_qkv                              heads=16, dh=64, seg=512, pm=16
  per (batch, segment, head): block-causal attention with 16 persistent
  memory tokens prepended to k/v, softmax, out = attn @ v
  out @ w_out                                  [2, 4096, 1024]

Sharding: data-parallel over the 16 (batch, segment) units; 2 contiguous
segments (1024 tokens) per core, full weights broadcast to all 8 cores.

Optimizations over the f32r baseline (sim ~294us -> ~187us, PE 87% busy):
- bf16 data path (weights, hT, qkT, v, p, aoT — PSUM stays f32): halves
  weight DMA, removes f32r small-block penalties, 2x DVE on 16-bit ops.
- pm scores: one [32,512] matmul per ot (both heads via zero-padded pmkt
  halves), 3 PSUM tiles per segment at bases 0/32/64, exp'd 6 heads/op.
- 3 exps per (seg,head) (512/384/384 packing) instead of 5.
- denominator rows collected per 8-head group via a dependency-tracked
  DRAM-tile round-trip (2 DMAs), normalization matmuls deferred one group
  so PE never waits on the reciprocal chain.
- issue-order software pipelining: deferred tt4-7 transposes inside the
  first projection chunk, D(seg0) heads interleaved 1:1 with B(seg1) units,
  D(seg1) heads interleaved with the first out-projection units; single
  shared PSUM pool (exactly 8 banks).
- engine balance: PSUM evacuation split DVE/ACT (Pool cannot touch PSUM),
  tri masks partly on Pool, DMA queues split SP/ACT with x loads leading.

Per-core pipeline:
  A: load x rows, sum(x^2) via ACT Square+accum, h_bf16 = x*rs via ACT
     copy-scale, PE-transpose (bf16) to hT[d, t], batched PSUM->SBUF copies
  B: qT/kT = W'.T @ h.T per 128-feature tile (q pre-scaled by dh^-1/2)
  C: v rows = h @ W'v interleaved with ones column (for denominators)
  D: per (seg, head): simT[j,i] = k.T @ q into 3 PSUM tiles, exp (ACT),
     causal tri-mask on diagonal 128-blocks (DVE), PV via [v|1] -> attnT +
     denominator row in PSUM; denominators collected per 8-head group via
     one DMA, reciprocal, broadcast via one-hot mask matmul, normalize aoT
  E: out rows = aoT.T @ w_out, PSUM->SBUF copy, batched DMA to DRAM
"""

import sys

sys.path.insert(0, "/opt/trn_rl_repo")

from contextlib import ExitStack

import numpy as np

import concourse.bass as bass
import concourse.mybir as mybir
import concourse.tile as tile
from concourse import bacc
from concourse.bass_utils import run_bass_kernel_spmd

F32 = mybir.dt.float32
BF16 = mybir.dt.bfloat16
I16 = mybir.dt.int16
AF = mybir.ActivationFunctionType

# Schraudolph fast-exp in bf16: exp(x) ~= bitcast_bf16(i16(x * 2^7/ln2 +
# (127 - c)*2^7)) with c ~= 0.0439 minimizing RMS error (~1.5% per weight;
# softmax renormalization over hundreds of keys suppresses it well below
# the 2e-2 output tolerance). Computed on DVE to offload the ACT engine.
EXP_A = 128.0 / float(np.log(2.0))
EXP_B = (127.0 - 0.0439) * 128.0

B, S, D = 2, 4096, 1024
HEADS, DH, SEG, PM = 16, 64, 512, 16
INNER = HEADS * DH          # 1024
NCORES = 8
TOK = (B * S) // NCORES     # 1024 tokens per core
NSEG = TOK // SEG           # 2 segments per core
TT = TOK // 128             # 8 token tiles
DT = D // 128               # 8 d tiles
NI2 = INNER // 128          # 8 inner tiles
EPS = 1e-6


def build_bass(repeat=1, stop="full"):
    # stop: "a" (norm+transpose), "abc" (+projections), "abcd" (+attention),
    #       "full" (+out projection). Partial builds are for phase timing only.
    nc = bacc.Bacc("TRN2", target_bir_lowering=False, debug=False)

    x_d = nc.dram_tensor("x", [TOK, D], F32, kind="ExternalInput")
    # host-pretiled so DMAs land as contiguous partition lines
    wqk_d = nc.dram_tensor("w_qk", [128, 2 * NI2, DT, 128], BF16, kind="ExternalInput")
    wv_d = nc.dram_tensor("w_v", [128, DT, INNER], BF16, kind="ExternalInput")
    wout_d = nc.dram_tensor("w_out", [128, NI2, D], BF16, kind="ExternalInput")
    pmkt_d = nc.dram_tensor("pm_kt", [128, NI2, 2 * PM], BF16, kind="ExternalInput")
    pmvo_d = nc.dram_tensor("pm_vo", [128, HEADS, DH + 1], BF16, kind="ExternalInput")
    tri_d = nc.dram_tensor("tri", [128, 128], BF16, kind="ExternalInput")
    ident_d = nc.dram_tensor("ident", [128, 128], BF16, kind="ExternalInput")
    hmask_d = nc.dram_tensor("hmask", [8, 2, NI2, 128], BF16, kind="ExternalInput")
    o_d = nc.dram_tensor("o", [TOK, D], F32, kind="ExternalOutput")

    with tile.TileContext(nc) as tc:
     for _rep in range(repeat):
      with ExitStack() as octx:
        # pools that live for the whole kernel
        consts = octx.enter_context(tc.tile_pool(name="consts", bufs=1))
        big = octx.enter_context(tc.tile_pool(name="big", bufs=1))

        # const loads are interleaved into the phase-A DMA schedule
        ident_sb = consts.tile([128, 128], BF16)
        tri_sb = consts.tile([128, 128], BF16)
        pmkt_sb = consts.tile([128, NI2, 2 * PM], BF16)
        pmvo_sb = consts.tile([128, HEADS, DH + 1], BF16)
        hmask_sb = consts.tile([8, 2, NI2, 128], BF16)
        eps_sb = consts.tile([128, 1], F32)
        nc.vector.memset(eps_sb[:], EPS)

        # resident weights (bf16); DMAs issued inside phase A so the x loads
        # (which gate all compute) go out first
        wqk_sb = big.tile([128, 2 * NI2, DT, 128], BF16)
        wv_sb = big.tile([128, DT, INNER], BF16)
        wo_sb = big.tile([128, NI2, D], BF16)

        # hT[p, db, t]: d = db*128 + p (bf16, rmsnorm applied)
        hT = big.tile([128, DT, TOK], BF16)
        # qkT[p, ot, t]: feature o = ot*128 + p; ot<8 -> q (pre-scaled), else k
        qkT = big.tile([128, 2 * NI2, TOK], BF16)
        # v[j_part, t_tile, head, dh+1]; column 64 holds ones (denominator)
        v_sb = big.tile([128, TT, HEADS, DH + 1], BF16)
        nc.vector.memset(v_sb[:, :, :, DH : DH + 1], 1.0)
        # unnormalized attnT output
        aoT = big.tile([128, NI2, TOK], BF16)

        # ---- Phase A: rmsnorm + transpose -> hT
        # transposes for tt4-7 are deferred into closures interleaved with
        # the first projection chunk so PE is never queued behind late tiles
        with ExitStack() as mctx:
            xh_pool = mctx.enter_context(tc.tile_pool(name="xh", bufs=3))
            stat_pool = mctx.enter_context(tc.tile_pool(name="stat", bufs=4))
            ps = mctx.enter_context(tc.tile_pool(name="ps", bufs=2, space="PSUM"))
            a_tp_units = []

            def a_transpose(tt, h_t):
                # PSUM comes from the shared [128,512] f32 slot, viewed bf16
                for dq in range(2):      # 4 transposes batched per copy
                    pt32 = ps.tile([128, 512], F32, tag="mm", name="pt32")
                    p_tr = pt32.bitcast(BF16)[:, 0:512].rearrange(
                        "p (j c) -> p j c", c=128
                    )
                    for j in range(4):
                        db = dq * 4 + j
                        nc.tensor.transpose(
                            p_tr[:, j, :], h_t[:, db * 128 : (db + 1) * 128], ident_sb[:]
                        )
                    dst = hT[:, dq * 4 : (dq + 1) * 4, tt * 128 : (tt + 1) * 128]
                    if dq == 0:
                        nc.vector.tensor_copy(dst, p_tr[:])
                    else:
                        nc.scalar.copy(dst, p_tr[:])

            # wqk chunks are interleaved with the late x loads on SP so
            # neither the first B units nor the tt4-7 transposes stall
            wqk_sched = {4: [0, 1], 5: [2, 3], 6: [4, 5], 7: [6, 7]}

            for tt in range(TT):
                x_t = xh_pool.tile([128, D], F32, tag="x", bufs=4)
                # x0/x1 lead the SP queue (no act-table load in front); x2
                # goes via ACT; the rest share SP with the weight chunks
                if tt == 2:
                    nc.scalar.dma_start(x_t[:], x_d[tt * 128 : (tt + 1) * 128, :])
                else:
                    nc.sync.dma_start(x_t[:], x_d[tt * 128 : (tt + 1) * 128, :])
                if tt == 0:
                    nc.sync.dma_start(ident_sb[:], ident_d[:])
                sq = stat_pool.tile([128, 1], F32, tag="sq")
                # scr is a write-only dummy (the accumulate is the product);
                # fp8 keeps its SBUF footprint minimal
                scr = xh_pool.tile([128, D], mybir.dt.float8e4, tag="scr", bufs=2)
                if tt % 2 == 1:
                    nc.scalar.activation(scr[:], x_t[:], AF.Square, accum_out=sq[:])
                else:
                    nc.vector.scalar_tensor_tensor(
                        scr[:], x_t[:], 1.0, x_t[:],
                        mybir.AluOpType.mult, mybir.AluOpType.mult,
                        accum_out=sq[:],
                    )
                s_t = stat_pool.tile([128, 1], F32, tag="s")
                nc.scalar.activation(s_t[:], sq[:], AF.Sqrt, bias=eps_sb[:], scale=1.0 / D)
                rs_t = stat_pool.tile([128, 1], F32, tag="rs")
                nc.vector.reciprocal(rs_t[:], s_t[:])
                h_t = xh_pool.tile([128, D], BF16, tag="h", bufs=5)
                if tt % 2 == 0:
                    nc.scalar.activation(h_t[:], x_t[:], AF.Copy, scale=rs_t[:])
                else:
                    nc.vector.tensor_scalar_mul(h_t[:], x_t[:], rs_t[:])
                if tt < 4:
                    a_transpose(tt, h_t)
                else:
                    a_tp_units.append(lambda tt=tt, h_t=h_t: a_transpose(tt, h_t))
                for c in wqk_sched.get(tt, []):
                    nc.sync.dma_start(
                        wqk_sb[:, 2 * c : 2 * (c + 1)],
                        wqk_d[:, 2 * c : 2 * (c + 1)],
                    )
                if tt == TT - 1:
                    nc.sync.dma_start(tri_sb[:], tri_d[:])
                    nc.sync.dma_start(pmkt_sb[:], pmkt_d[:])
                    nc.sync.dma_start(pmvo_sb[:], pmvo_d[:])
                    nc.sync.dma_start(hmask_sb[:], hmask_d[:])

            # ---- Phases B, C, D interleaved per segment; shared PSUM pool
            p_pool = mctx.enter_context(tc.tile_pool(name="p", bufs=3))
            pm_pool = mctx.enter_context(tc.tile_pool(name="ppm", bufs=3))
            den_pool = mctx.enter_context(tc.tile_pool(name="den", bufs=2))
            stage_pool = mctx.enter_context(tc.tile_pool(name="stage", bufs=1))
            dram_pool = mctx.enter_context(
                tc.tile_pool(name="dram", bufs=2, space="DRAM")
            )

            def b_unit(tch, ot):
                # one q/k feature tile for token chunk tch (== segment tch)
                pmm = ps.tile([128, 512], F32, tag="mm", name="pmm")
                for db in range(DT):
                    nc.tensor.matmul(
                        pmm[:],
                        wqk_sb[:, ot, db, :],
                        hT[:, db, tch * 512 : (tch + 1) * 512],
                        start=(db == 0),
                        stop=(db == DT - 1),
                    )
                dst = qkT[:, ot, tch * 512 : (tch + 1) * 512]
                # tch1 units run while seg0 attention saturates ACT with
                # exps, so their copies all go to DVE
                if tch == 0 and ot % 2 == 1:
                    nc.scalar.copy(dst, pmm[:])
                else:
                    nc.vector.tensor_copy(dst, pmm[:])

            def c_unit(tt, och):
                # one 512-wide chunk of v rows for token tile tt
                pmm = ps.tile([128, 512], F32, tag="mm", name="pmm")
                for db in range(DT):
                    nc.tensor.matmul(
                        pmm[:],
                        hT[:, db, tt * 128 : (tt + 1) * 128],
                        wv_sb[:, db, och * 512 : (och + 1) * 512],
                        start=(db == 0),
                        stop=(db == DT - 1),
                    )
                dst = v_sb[:, tt, och * 8 : (och + 1) * 8, 0:DH]
                src = pmm[:].rearrange("p (h o) -> p h o", o=DH)
                if och == 0:
                    nc.scalar.copy(dst, src)
                else:
                    nc.vector.tensor_copy(dst, src)

            def b_units(tch):
                return [
                    (lambda ot=ot: b_unit(tch, ot)) for ot in range(2 * NI2)
                ]

            def c_units(tch):
                return [
                    (lambda tt=tt, och=och: c_unit(tt, och))
                    for tt in range(4 * tch, 4 * tch + 4)
                    for och in range(INNER // 512)
                ]

            def phase_d_pm(seg):
                # pm scores for all 16 heads: one [32, 512] matmul per ot
                # (both heads of the ot via zero-padded halves of pmkt),
                # stacked 3 ots per PSUM tile at partition bases 0/32/64
                p_pms = []
                for c in range(3):          # ots [0,1,2], [3,4,5], [6,7]
                    ots = range(3 * c, min(3 * c + 3, NI2))
                    n = 32 * len(ots)
                    psm = ps.tile([128, 512], F32, tag="mm")
                    for i, ot in enumerate(ots):
                        nc.tensor.matmul(
                            psm[32 * i : 32 * i + 32, :],
                            pmkt_sb[:, ot, :],
                            qkT[:, ot, seg * 512 : (seg + 1) * 512],
                            start=True,
                            stop=True,
                        )
                    p_pm = pm_pool.tile([128, 512], BF16, tag="ppm")
                    nc.scalar.activation(p_pm[0:n, :], psm[0:n, :], AF.Exp)
                    p_pms.append(p_pm)
                return p_pms

            def pm_ops(p_pms, h):
                # 32-row slice at base 0/32/64 covering both heads of the ot;
                # pmvo rows for the other head are zero, so the contraction
                # picks out only head h's pm scores
                ot = h // 2
                base = 32 * (ot % 3)
                lhsT = pmvo_sb[base : base + 32, h, :]
                rhs = p_pms[ot // 3][base : base + 32, :]
                return lhsT, rhs

            def d_head(seg, h, p_pms, dstage):
                    pb = (h % 2) * 64
                    ot = h // 2
                    q_ap = qkT[pb : pb + 64, ot, seg * 512 : (seg + 1) * 512]
                    k_ap = qkT[pb : pb + 64, NI2 + ot, seg * 512 : (seg + 1) * 512]

                    ps0 = ps.tile([128, 512], F32, tag="s512")
                    nc.tensor.matmul(ps0[:], k_ap[:, 0:128], q_ap[:], start=True, stop=True)
                    ps1 = ps.tile([128, 384], F32, tag="s384")
                    nc.tensor.matmul(
                        ps1[:], k_ap[:, 128:256], q_ap[:, 128:512], start=True, stop=True
                    )
                    ps23 = ps.tile([128, 384], F32, tag="s384")
                    nc.tensor.matmul(
                        ps23[:, 0:256], k_ap[:, 256:384], q_ap[:, 256:512],
                        start=True, stop=True,
                    )
                    nc.tensor.matmul(
                        ps23[:, 256:384], k_ap[:, 384:512], q_ap[:, 384:512],
                        start=True, stop=True,
                    )
                    p0 = p_pool.tile([128, 512], BF16, tag="p0")
                    nc.scalar.activation(p0[:], ps0[:], AF.Exp)
                    p1 = p_pool.tile([128, 384], BF16, tag="p1")
                    nc.scalar.activation(p1[:], ps1[:], AF.Exp)
                    p23 = p_pool.tile([128, 384], BF16, tag="p23")
                    nc.scalar.activation(p23[:], ps23[:], AF.Exp)
                    # causal mask on the four diagonal 128-blocks; two on
                    # Pool (SBUF-only engine), two on DVE
                    nc.gpsimd.tensor_mul(p0[:, 0:128], p0[:, 0:128], tri_sb[:])
                    nc.gpsimd.tensor_mul(p1[:, 0:128], p1[:, 0:128], tri_sb[:])
                    nc.vector.tensor_mul(p23[:, 0:128], p23[:, 0:128], tri_sb[:])
                    nc.vector.tensor_mul(p23[:, 256:384], p23[:, 256:384], tri_sb[:])

                    # PV with ones column: rows 0..63 attnT, row 64 denominator
                    pv = ps.tile([DH + 1, 512], F32, tag="pv")
                    s4 = seg * 4
                    pm_lhsT, pm_rhs = pm_ops(p_pms, h)
                    nc.tensor.matmul(pv[:], pm_lhsT, pm_rhs, start=True, stop=False)
                    nc.tensor.matmul(
                        pv[:], v_sb[:, s4, h, :], p0[:], start=False, stop=False
                    )
                    nc.tensor.matmul(
                        pv[:, 128:512], v_sb[:, s4 + 1, h, :], p1[:],
                        start=False, stop=False,
                    )
                    nc.tensor.matmul(
                        pv[:, 256:512], v_sb[:, s4 + 2, h, :], p23[:, 0:256],
                        start=False, stop=False,
                    )
                    nc.tensor.matmul(
                        pv[:, 384:512], v_sb[:, s4 + 3, h, :], p23[:, 256:384],
                        start=False, stop=True,
                    )
                    # unnormalized attnT into aoT slice; DVE carries these
                    # (ACT is exp-saturated during attention windows)
                    ao_dst = aoT[pb : pb + 64, h // 2, seg * 512 : (seg + 1) * 512]
                    dst_dst = dstage[DH : DH + 1, (h % 8) * 512 : (h % 8) * 512 + 512]
                    nc.vector.tensor_copy(ao_dst, pv[0:DH, :])
                    if h % 2 == 0:
                        nc.scalar.copy(dst_dst, pv[DH : DH + 1, :])
                    else:
                        nc.vector.tensor_copy(dst_dst, pv[DH : DH + 1, :])

            def d_group_end(seg, g, dstage, rb_queue):
                # batch the 8 denominator rows across partitions via a DRAM
                # round-trip. The scratch is a DRAM-space tile so the write
                # and readback DMAs get a real dependency edge (raw internal
                # dram tensors are not tracked -> HW race). The final group
                # goes via the ACT queue: SP is draining the out-projection
                # writes at that point.
                dma = nc.scalar if seg == NSEG - 1 and g == 1 else nc.sync
                den_dram = dram_pool.tile([1, 8 * 512], BF16, tag="dend")
                dma.dma_start(den_dram[:], dstage[DH : DH + 1, :])
                den_g = den_pool.tile([8, 512], BF16, tag="deng")
                dma.dma_start(
                    den_g[:], den_dram[:].rearrange("o (h t) -> (o h) t", h=8)
                )
                rec_g = den_pool.tile([8, 512], BF16, tag="recg")
                with nc.allow_low_precision(reason="bf16 reciprocal feeds bf16 matmul"):
                    nc.vector.reciprocal(rec_g[:], den_g[:])

                def flush_rb():
                    for ti2 in range(4 * g, 4 * g + 4):
                        rb = ps.tile([128, 512], F32, tag="mm", name="rb")
                        nc.tensor.matmul(
                            rb[:],
                            hmask_sb[:, g, ti2, :],
                            rec_g[:],
                            start=True,
                            stop=True,
                        )
                        ao_ap = aoT[:, ti2, seg * 512 : (seg + 1) * 512]
                        nc.vector.tensor_mul(ao_ap, ao_ap, rb[:])

                rb_queue.append(flush_rb)

            def d_units(seg, rb_queue):
                # 16 per-head closures; pm scores are emitted by the first
                # unit, denominator bookkeeping rides the 8th and 16th
                state = {}

                def unit(h):
                    if h == 0:
                        state["p_pms"] = phase_d_pm(seg)
                        state["dst"] = stage_pool.tile(
                            [DH + 1, 8 * 512], BF16, tag="dst", name="dstage"
                        )
                    d_head(seg, h, state["p_pms"], state["dst"])
                    if h == 7:
                        d_group_end(seg, 0, state["dst"], rb_queue)
                        state["dst"] = stage_pool.tile(
                            [DH + 1, 8 * 512], BF16, tag="dst", name="dstage"
                        )
                    elif h == 15:
                        d_group_end(seg, 1, state["dst"], rb_queue)

                return [(lambda h=h: unit(h)) for h in range(HEADS)]

            def e_units():
                # out projection per (token tile, 512-wide output chunk)
                o_pool = mctx.enter_context(tc.tile_pool(name="o", bufs=2))

                def unit(tt, ech):
                    o_sb = o_pool.tile([128, 512], F32, tag="osb", name="o_sb")
                    pso = ps.tile([128, 512], F32, tag="mm", name="pso")
                    for ti2 in range(NI2):
                        nc.tensor.matmul(
                            pso[:],
                            aoT[:, ti2, tt * 128 : (tt + 1) * 128],
                            wo_sb[:, ti2, ech * 512 : (ech + 1) * 512],
                            start=(ti2 == 0),
                            stop=(ti2 == NI2 - 1),
                        )
                    if ech == 0:
                        nc.scalar.copy(o_sb[:], pso[:])
                        nc.scalar.dma_start(
                            o_d[tt * 128 : (tt + 1) * 128, 0:512], o_sb[:]
                        )
                    else:
                        nc.vector.tensor_copy(o_sb[:], pso[:])
                        nc.sync.dma_start(
                            o_d[tt * 128 : (tt + 1) * 128, 512:1024], o_sb[:]
                        )

                return [
                    (lambda tt=tt, ech=ech: unit(tt, ech))
                    for tt in range(TT)
                    for ech in range(D // 512)
                ]

            def interleave(xs, ys, nx, ny):
                # emit xs and ys round-robin, nx of xs then ny of ys
                xs, ys = list(xs), list(ys)
                while xs or ys:
                    for _ in range(nx):
                        if xs:
                            xs.pop(0)()
                    for _ in range(ny):
                        if ys:
                            ys.pop(0)()

            def load_wv():
                # deferred out of the bandwidth-critical startup window (the
                # x/wqk burst); SP is idle mid-B0 and C0 needs wv only later
                for c in range(2):
                    nc.sync.dma_start(
                        wv_sb[:, 4 * c : 4 * (c + 1)],
                        wv_d[:, 4 * c : 4 * (c + 1)],
                    )

            rb_queue = []
            if stop == "a":
                for u in a_tp_units:
                    u()
            elif stop == "abc":
                load_wv()
                interleave(b_units(0), a_tp_units, 3, 1)
                for u in c_units(0) + b_units(1) + c_units(1):
                    u()
            else:
                emit_e = stop == "full"
                # seg0 q/k projection, with the deferred tt4-7 transposes
                # slotted between ot units; then seg0 v projection
                b0 = b_units(0)
                interleave(b0[:6], a_tp_units[:2], 3, 1)
                load_wv()
                interleave(b0[6:], a_tp_units[2:], 3, 1)
                for u in c_units(0):
                    u()
                # seg0 attention overlapped with seg1 q/k projection
                interleave(d_units(0, rb_queue), b_units(1), 1, 1)
                # seg1 v projection; seg0 normalization interleaved. The
                # out-projection weights load here, well clear of the
                # bandwidth-critical startup window (E needs them much later)
                for c in range(2):
                    nc.sync.dma_start(
                        wo_sb[:, 4 * c : 4 * (c + 1)],
                        wout_d[:, 4 * c : 4 * (c + 1)],
                    )
                c1 = c_units(1)
                for i, u in enumerate(c1):
                    u()
                    if i == 3 or i == 7:
                        rb_queue.pop(0)()
                # seg1 attention overlapped with the start of the out
                # projection (token tiles 0-3 need only seg0); the long E
                # tail gives the seg1 denominator round-trips time to land
                ev = e_units() if emit_e else []
                interleave(d_units(1, rb_queue), ev[:4], 4, 1)
                if rb_queue:
                    rb_queue.pop(0)()
                for u in ev[4:8]:
                    u()
                while rb_queue:
                    rb_queue.pop(0)()
                # last two units swapped so the final DRAM writes drain on
                # both HWDGE queues in parallel
                tail = ev[8:]
                tail[-1], tail[-2] = tail[-2], tail[-1]
                for u in tail:
                    u()

    nc.compile()
    return nc


_NC_CACHE = None


def _get_nc():
    global _NC_CACHE
    if _NC_CACHE is None:
        _NC_CACHE = build_bass()
    return _NC_CACHE


class _Runner:
    """Compile the Bass program once into a sharded jitted callable over the
    8 NeuronCores; reuse it for every kernel() invocation."""

    def __init__(self, nc):
        import jax
        from jax.sharding import Mesh, PartitionSpec
        from jax.experimental.shard_map import shard_map
        from concourse import bass2jax

        bass2jax.install_neuronx_cc_hook()
        self.nc = nc
        pname = nc.partition_id_tensor.name if nc.partition_id_tensor else None
        in_names, out_names, out_avals, self.zero_shapes = [], [], [], []
        for alloc in nc.m.functions[0].allocations:
            if not isinstance(alloc, mybir.MemoryLocationSet):
                continue
            name = alloc.memorylocations[0].name
            if alloc.kind == "ExternalInput":
                if name != pname:
                    in_names.append(name)
            elif alloc.kind == "ExternalOutput":
                out_names.append(name)
                shape = tuple(alloc.tensor_shape)
                dtype = mybir.dt.np(alloc.dtype)
                out_avals.append(jax.core.ShapedArray(shape, dtype))
                self.zero_shapes.append((shape, dtype))
        self.in_names, self.out_names = in_names, out_names
        all_in = in_names + out_names + ([pname] if pname else [])

        def _body(*args):
            operands = list(args)
            if pname is not None:
                operands.append(bass2jax.partition_id_tensor())
            return tuple(
                bass2jax._bass_exec_p.bind(
                    *operands,
                    out_avals=tuple(out_avals),
                    in_names=tuple(all_in),
                    out_names=tuple(out_names),
                    lowering_input_output_aliases=(),
                    sim_require_finite=True,
                    sim_require_nnan=True,
                    nc=nc,
                )
            )

        devices = jax.devices()[:NCORES]
        self.mesh = Mesh(np.asarray(devices), ("core",))
        self.sharding = jax.sharding.NamedSharding(self.mesh, PartitionSpec("core"))
        n_params = len(in_names)
        donate = tuple(range(n_params, n_params + len(out_names)))
        self.sharded = jax.jit(
            shard_map(
                _body,
                mesh=self.mesh,
                in_specs=(PartitionSpec("core"),) * (n_params + len(out_names)),
                out_specs=(PartitionSpec("core"),) * len(out_names),
                check_rep=False,
            ),
            donate_argnums=donate,
            keep_unused=True,
        )
        self._jax = jax

    def device_inputs(self, in_maps):
        concat = [
            np.concatenate([np.asarray(m[nm]) for m in in_maps], axis=0)
            for nm in self.in_names
        ]
        return [self._jax.device_put(a, self.sharding) for a in concat]

    def zeros(self):
        return [
            self._jax.device_put(
                np.zeros((NCORES * s[0], *s[1:]), d), self.sharding
            )
            for s, d in self.zero_shapes
        ]

    def __call__(self, dev_in):
        outs = self.sharded(*dev_in, *self.zeros())
        for o in outs:
            o.block_until_ready()
        return outs


_RUNNER = None


def _get_runner():
    global _RUNNER
    if _RUNNER is None:
        _RUNNER = _Runner(_get_nc())
    return _RUNNER


def make_in_maps(x, gamma, w_qkv, w_out, pm_k, pm_v):
    bf16 = mybir.dt.np(BF16)
    x = np.asarray(x, dtype=np.float32).reshape(B * S, D)
    gamma = np.asarray(gamma, dtype=np.float32)
    w_qkv = np.asarray(w_qkv, dtype=np.float32)
    w_out = np.asarray(w_out, dtype=np.float32)
    pm_k = np.asarray(pm_k, dtype=np.float32)
    pm_v = np.asarray(pm_v, dtype=np.float32)

    w = w_qkv * gamma[:, None]                       # fold gamma into the projection
    scale = DH ** -0.5
    w_qk = np.concatenate([w[:, :INNER] * scale, w[:, INNER : 2 * INNER]], axis=1)
    # [D, 2*INNER] with d = db*128 + dc, o = ot*128 + oc.
    # lhsT layout: [dc (partitions, contract), ot, db, oc (out rows)]
    w_qk = np.ascontiguousarray(
        w_qk.reshape(DT, 128, 2 * NI2, 128).transpose(1, 2, 0, 3)
    ).astype(bf16)

    # w_v: [D, INNER] -> [p, db, o] with d = db*128 + p
    w_v = np.ascontiguousarray(
        w[:, 2 * INNER :].reshape(DT, 128, INNER).transpose(1, 0, 2)
    ).astype(bf16)
    # w_out: [INNER, D] -> [p, ib, e] with i = ib*128 + p
    w_o = np.ascontiguousarray(
        w_out.reshape(NI2, 128, D).transpose(1, 0, 2)
    ).astype(bf16)

    # q is pre-scaled by dh^-1/2, so pm keys are used unscaled.
    # Both heads of an ot in one [128, 32] lhsT: head 2ot on partitions 0:64
    # cols 0:16, head 2ot+1 on partitions 64:128 cols 16:32 (zeros elsewhere
    # make the full-128-partition contraction per-head exact).
    pm_kt = np.zeros((128, NI2, 2 * PM), dtype=np.float32)
    for h in range(HEADS):
        half = h % 2
        pm_kt[64 * half : 64 * half + 64, h // 2, 16 * half : 16 * half + 16] = (
            pm_k[h].T
        )
    # pmvo replicated at partition bases 32*(ot%3): head h occupies rows
    # 32*(ot%3) + 16*(h%2) .. +16 (values + ones column), zeros elsewhere
    pm_vo = np.zeros((128, HEADS, DH + 1), dtype=np.float32)
    for h in range(HEADS):
        r0 = 32 * ((h // 2) % 3) + 16 * (h % 2)
        pm_vo[r0 : r0 + PM, h, :DH] = pm_v[h]
        pm_vo[r0 : r0 + PM, h, DH] = 1.0

    r = np.arange(128)
    tri = (r[:, None] <= r[None, :]).astype(np.float32)
    ident = np.eye(128, dtype=np.float32)
    # hmask[j, g, ti2, m] = 1 iff head 8g+j owns partition m of inner tile ti2
    hmask = np.zeros((8, 2, NI2, 128), dtype=np.float32)
    for ti2 in range(NI2):
        for m in range(128):
            h = (ti2 * 128 + m) // DH
            hmask[h % 8, h // 8, ti2, m] = 1.0

    shared = {
        "w_qk": w_qk,
        "w_v": w_v,
        "w_out": w_o,
        "pm_kt": pm_kt.astype(bf16),
        "pm_vo": pm_vo.astype(bf16),
        "tri": tri.astype(bf16),
        "ident": ident.astype(bf16),
        "hmask": hmask.astype(bf16),
    }
    return [
        {"x": np.ascontiguousarray(x[c * TOK : (c + 1) * TOK]), **shared}
        for c in range(NCORES)
    ]


def kernel(x, gamma, w_qkv, w_out, pm_k, pm_v):
    runner = _get_runner()
    in_maps = make_in_maps(x, gamma, w_qkv, w_out, pm_k, pm_v)
    outs = runner(runner.device_inputs(in_maps))
    out = np.asarray(outs[0])          # [NCORES*TOK, D] global row-sharded
    return out.reshape(B, S, D)


if __name__ == "__main__":
    rng = np.random.default_rng(0)
    ins = {
        "x": rng.standard_normal((B, S, D), dtype=np.float32),
        "gamma": np.ones(D, dtype=np.float32),
        "w_qkv": (rng.standard_normal((D, 3 * INNER), dtype=np.float32) * D**-0.5),
        "w_out": (rng.standard_normal((INNER, D), dtype=np.float32) * INNER**-0.5),
        "pm_k": (rng.standard_normal((HEADS, PM, DH), dtype=np.float32) * 0.02),
        "pm_v": (rng.standard_normal((HEADS, PM, DH), dtype=np.float32) * 0.02),
    }
    out = kernel(**ins)
    print("out", out.shape, out.dtype, np.abs(out).mean())
